# revision 22
# baseline (speedup 1.0000x reference)
"""MoE MLP (E=4, top-2 routing) Trainium2 kernel, 8 NeuronCores.

Sharding (expert x F-half tensor parallel): core c handles expert e = c//2
and F-half = c%2 (columns [half*2048, (half+1)*2048) of w1, matching rows of
w2).  Each core computes, for every token routed to its expert,
    y_half = gelu(x @ w1[e, :, half]) @ w2[e, half, :]
with tokens living in the matmul FREE dimension, so the token count needs no
128-padding -- the compiled program uses C = max_e n_e token columns.  The
host gathers the two halves, scales by the routing probs, scatters back to
token order and adds the residual.

Precision: matmuls run in fp8e4m3 with MatmulPerfMode.DoubleRow (two
contraction k-tiles per instruction at 0.5 cycles/row).  Weights use
error-compensated quantization: w*32 ~= q8(w*32) + q8(w*32 - q8(w*32)), the
residual folded in as extra DoubleRow accumulation terms, so only the
activation quantization (x and gelu output) contributes first-order error.
Optionally (T1=3) the x residual dx = q8(x - q8(x)) is compensated too
(host-computed, zero extra device passes).  The *32 scaling keeps weight
values out of the fp8 subnormal range; it is undone for free via the gelu
activation scale (mm1) and the PSUM->SBUF copy scale (mm2).

This covers ANY routing map: every (token, expert) pair with routing_map
True lands in expert e's token list; tokens with 0 experts just pass the
residual through.  Capacity per expert is the full token count, so no
fallback path is needed.
"""
import math
import sys

import numpy as np

try:
    import concourse.bass as bass  # noqa: F401
except Exception:
    sys.path.insert(0, "/opt/trn_rl_repo")

import ml_dtypes

import concourse.bacc as bacc
import concourse.bass as bass
import concourse.mybir as mybir
import concourse.tile as tile
from concourse.bass_utils import run_bass_kernel_spmd

S, B, H, F, E = 1024, 2, 1024, 4096, 4
T = S * B
N_CORES = 8
FH = F // 2     # 2048, per-core F slice
NH = H // 128   # 8 h-chunks
NFH = FH // 128  # 16 f-chunks per core
F8_DT = mybir.dt.float8e4
F8_NP = ml_dtypes.float8_e4m3
WSCALE = 32.0   # weight pre-scale, undone on device

# mm1 terms: (w1s8, x8), (dw1s8, x8) [, (w1s8, dx8) if T1 == 3]
# mm2 terms: (w2s8, a8), (dw2s8, a8)
T1 = 2
T2 = 2

_NC_CACHE = {}


def _pieces(C):
    """Split C token columns into pieces of <= 512 (PSUM bank limit).

    First piece exactly 512 when possible (fp8 DMA runs >= 512B avoid the
    2x small-transfer latency penalty); the LAST piece is small (but >= 160
    so matmul exec stays above the 25ns sequencer dispatch) to shorten the
    post-matmul tail.
    """
    if C <= 512:
        widths = [C]
    else:
        last = max(160, min(512, round(C * 0.16)))
        mid = C - 512 - last
        if mid <= 0:
            widths = [512, C - 512] if C - 512 >= 160 else [C - 160, 160]
        else:
            P = math.ceil(mid / 512)
            base, rem = divmod(mid, P)
            widths = [512] + [base + 1] * rem + [base] * (P - rem) + [last]
    offs, o = [], 0
    for w in widths:
        offs.append(o)
        o += w
    return list(zip(offs, widths))


def _build_nc(C):
    key = (C,)
    if key in _NC_CACHE:
        return _NC_CACHE[key]
    pieces = _pieces(C)
    P = len(pieces)
    f32 = mybir.dt.float32
    gelu = mybir.ActivationFunctionType.Gelu
    dr = mybir.MatmulPerfMode.DoubleRow
    nc = bacc.Bacc("TRN2", target_bir_lowering=False, debug=False,
                   num_devices=N_CORES)
    xt_d = nc.declare_dram_parameter("xt8", [H, C], F8_DT, isOutput=False)
    if T1 == 3:
        dxt_d = nc.declare_dram_parameter("dxt8", [H, C], F8_DT,
                                          isOutput=False)
    # weights host-blocked, fq-major with the two quantization slots
    # interleaved, so one DMA covers a contiguous fq range for both slots
    # with >=1KB contiguous runs (runs < 512B pay 2x DMA latency).
    w1_d = nc.declare_dram_parameter("w1b", [NFH, 2, 128, NH, 128], F8_DT,
                                     isOutput=False)
    w2_d = nc.declare_dram_parameter("w2b", [NFH, 2, 128, H], F8_DT,
                                     isOutput=False)
    y_d = nc.declare_dram_parameter("y", [H, C], f32, isOutput=True)

    with tile.TileContext(nc) as tc:
        with (
            tc.tile_pool(name="resident", bufs=1) as rpool,
            tc.tile_pool(name="a", bufs=3) as apool,
            tc.tile_pool(name="y", bufs=2) as ypool,
            tc.tile_pool(name="pa", bufs=4, space="PSUM") as papool,
            tc.tile_pool(name="py", bufs=4, space="PSUM") as pypool,
        ):
            xt_sb = rpool.tile([128, NH, C], F8_DT, tag="xt")
            w1_sb = rpool.tile([128, NFH, 2, NH, 128], F8_DT, tag="w1")
            w2_sb = rpool.tile([128, NFH, 2, H], F8_DT, tag="w2")
            xt_ap = xt_d.ap().rearrange("(hc h) c -> h hc c", h=128)
            w1_ap = w1_d.ap().rearrange("q s h hc f -> h q s hc f")
            w2_ap = w2_d.ap().rearrange("q s f h -> f q s h")
            y_ap = y_d.ap().rearrange("(hc h) c -> h hc c", h=128)

            # Each dma_start holds the (serialized) HWDGE descriptor stage
            # ~625ns and the DMA engines for its transfer, so DMAs are few,
            # big, and issued in need order with geometric w1 head chunks so
            # the PE unblocks early.
            o0, w0 = pieces[0]
            if T1 == 3:
                dxt_sb = rpool.tile([128, NH, C], F8_DT, tag="dxt")
                dxt_ap = dxt_d.ap().rearrange("(hc h) c -> h hc c", h=128)

            def xt_piece(sb, ap, o, w, hcs):
                for (a, b) in hcs:
                    nc.sync.dma_start(sb[:, a:b, o:o + w],
                                      ap[:, a:b, o:o + w])

            def w_chunk(sb, ap, a, b):
                nc.sync.dma_start(sb[:, a:b], ap[:, a:b])

            xt_piece(xt_sb, xt_ap, o0, w0, [(0, 2)])
            nc.sync.dma_start(w1_sb[:, 0:1, 0], w1_ap[:, 0:1, 0])
            xt_piece(xt_sb, xt_ap, o0, w0, [(2, 4), (4, 8)])
            nc.sync.dma_start(w1_sb[:, 0:1, 1], w1_ap[:, 0:1, 1])
            if T1 == 3:
                xt_piece(dxt_sb, dxt_ap, o0, w0, [(0, 8)])
            w_chunk(w1_sb, w1_ap, 1, 2)
            w_chunk(w1_sb, w1_ap, 2, 3)
            w_chunk(w1_sb, w1_ap, 3, 4)
            w_chunk(w1_sb, w1_ap, 4, 6)
            w_chunk(w1_sb, w1_ap, 6, 8)
            w_chunk(w1_sb, w1_ap, 8, 10)
            if len(pieces) > 1:
                o, w = pieces[1]
                xt_piece(xt_sb, xt_ap, o, w, [(0, 8)])
                if T1 == 3:
                    xt_piece(dxt_sb, dxt_ap, o, w, [(0, 8)])
            w_chunk(w1_sb, w1_ap, 10, 12)
            w_chunk(w1_sb, w1_ap, 12, 14)
            w_chunk(w1_sb, w1_ap, 14, NFH)
            w_chunk(w2_sb, w2_ap, 0, 4)
            for (o, w) in pieces[2:]:
                xt_piece(xt_sb, xt_ap, o, w, [(0, 8)])
                if T1 == 3:
                    xt_piece(dxt_sb, dxt_ap, o, w, [(0, 8)])
            w_chunk(w2_sb, w2_ap, 4, 8)
            w_chunk(w2_sb, w2_ap, 8, 12)
            w_chunk(w2_sb, w2_ap, 12, NFH)

            # Warm the PE p-state during the DMA head: the cost model runs
            # the PE ~2x slower until it has been continuously busy for 3us,
            # so burn the input-DMA latency on self-contained dummy matmuls
            # over a memset tile instead of idling.
            warm_sb = rpool.tile([128, 2, 256], F8_DT, tag="warm")
            nc.vector.memset(warm_sb[:], 0)
            wpa = papool.tile([128, 256], f32, tag="pa", name="warm_pa")
            for _ in range(64):
                nc.tensor.matmul(wpa[:, :], warm_sb[:, :, 0:128],
                                 warm_sb[:, :, :], start=True, stop=True,
                                 perf_mode=dr)

            a_tiles = {}

            def mm1(p):
                off, W = pieces[p]
                a_sb = apool.tile([128, NFH, W], F8_DT, tag="a",
                                  name=f"a_{p}")
                a_tiles[p] = a_sb
                terms = [(0, xt_sb), (1, xt_sb)]
                if T1 == 3:
                    terms.append((0, dxt_sb))
                for fq in range(NFH):
                    pa = papool.tile([128, W], f32, tag="pa")
                    n = len(terms) * (NH // 2)
                    i = 0
                    for s, rhs in terms:
                        for j in range(NH // 2):
                            nc.tensor.matmul(
                                pa[:, :],
                                w1_sb[:, fq, s, 2 * j:2 * j + 2, :],
                                rhs[:, 2 * j:2 * j + 2, off:off + W],
                                start=(i == 0), stop=(i == n - 1),
                                perf_mode=dr)
                            i += 1
                    # psum holds 32*(x @ w1half); gelu(psum/32) -> fp8 a
                    nc.scalar.activation(a_sb[:, fq, :], pa[:, :], gelu,
                                         scale=1.0 / WSCALE)

            def mm2(p):
                off, W = pieces[p]
                last = (p == P - 1)
                a_sb = a_tiles.pop(p)
                y_sb = ypool.tile([128, NH, W], f32, tag="y", name=f"y_{p}")
                for hc in range(NH):
                    py = pypool.tile([128, W], f32, tag="py")
                    n = T2 * (NFH // 2)
                    i = 0
                    for s in range(T2):
                        for j in range(NFH // 2):
                            nc.tensor.matmul(
                                py[:, :],
                                w2_sb[:, 2 * j:2 * j + 2, s,
                                      hc * 128:(hc + 1) * 128],
                                a_sb[:, 2 * j:2 * j + 2, :],
                                start=(i == 0), stop=(i == n - 1),
                                perf_mode=dr)
                            i += 1
                    # psum holds 32*(a @ w2half); copy-with-scale undoes it
                    nc.vector.tensor_scalar_mul(y_sb[:, hc, :], py[:, :],
                                                1.0 / WSCALE)
                    # two y DMAs per piece (HWDGE is serialized, keep DMA
                    # count low, but don't clog the pipe right before the
                    # final small post-matmul DMA either).
                    if hc == 3:
                        nc.sync.dma_start(y_ap[:, :4, off:off + W],
                                          y_sb[:, :4, :])
                    elif hc == NH - 1:
                        nc.sync.dma_start(y_ap[:, 4:, off:off + W],
                                          y_sb[:, 4:, :])

            # Interleave so piece p's gelus fully overlap PE work, and the
            # PE never waits on the ACT engine at a piece boundary.
            mm1(0)
            for p in range(1, P):
                mm1(p)
                mm2(p - 1)
            mm2(P - 1)
    nc.compile()
    _NC_CACHE[key] = nc
    return nc


def _q8(v):
    return np.asarray(v, F8_NP)


def _block_w1(w):
    """[H, FH] -> [NFH, 128, NH, 128] (fq, h, hc, f)."""
    return w.reshape(NH, 128, NFH, 128).transpose(2, 1, 0, 3)


def kernel(hidden_states, mlp_residual, probs, routing_map, w1, w2,
           _trace=False):
    hidden_states = np.asarray(hidden_states, np.float32)
    mlp_residual = np.asarray(mlp_residual, np.float32)
    probs = np.asarray(probs, np.float32)
    routing_map = np.asarray(routing_map, bool)
    w1 = np.asarray(w1, np.float32)
    w2 = np.asarray(w2, np.float32)

    x = hidden_states.reshape(T, H)
    idx = [np.nonzero(routing_map[:, e])[0] for e in range(E)]
    C = max(1, max(len(i) for i in idx))

    nc = _build_nc(C)

    in_maps = []
    for c in range(N_CORES):
        e, half = divmod(c, 2)
        tok = idx[e]
        xtf = np.zeros((C, H), np.float32)
        if len(tok):
            xtf[:len(tok)] = x[tok]
        x8 = _q8(xtf)
        m = {"xt8": np.ascontiguousarray(x8.astype(np.float32).T).astype(
            F8_NP)}
        if T1 == 3:
            dx8 = _q8(xtf - x8.astype(np.float32))
            m["dxt8"] = np.ascontiguousarray(
                dx8.astype(np.float32).T).astype(F8_NP)
        w1s = w1[e, :, half * FH:(half + 1) * FH] * WSCALE
        w1s8 = _q8(w1s)
        dw1s8 = _q8(w1s - w1s8.astype(np.float32))
        # [NFH, 2, 128, NH, 128]: fq-major, quantization slots interleaved
        m["w1b"] = np.ascontiguousarray(np.stack(
            [_block_w1(w1s8.astype(np.float32)),
             _block_w1(dw1s8.astype(np.float32))], axis=1)).astype(F8_NP)
        w2s = w2[e, half * FH:(half + 1) * FH, :] * WSCALE
        w2s8 = _q8(w2s)
        dw2s8 = _q8(w2s - w2s8.astype(np.float32))
        # [NFH, 2, 128, H]
        m["w2b"] = np.ascontiguousarray(np.stack(
            [w2s8.astype(np.float32).reshape(NFH, 128, H),
             dw2s8.astype(np.float32).reshape(NFH, 128, H)],
            axis=1)).astype(F8_NP)
        in_maps.append(m)

    r = run_bass_kernel_spmd(nc, in_maps, list(range(N_CORES)), trace=_trace)

    out = mlp_residual.reshape(T, H).astype(np.float32).copy()
    for e in range(E):
        tok = idx[e]
        if len(tok) == 0:
            continue
        y = (np.asarray(r.results[2 * e]["y"][:, :len(tok)], np.float32)
             + np.asarray(r.results[2 * e + 1]["y"][:, :len(tok)],
                          np.float32))
        psel = probs[tok, e].astype(np.float32)
        out[tok] += (y * psel[None, :]).T
    result = out.reshape(S, B, H)
    if _trace:
        return result, r
    return result


# revision 23
# speedup vs baseline: 1.0189x; 1.0189x over previous
"""MoE MLP (E=4, top-2 routing) Trainium2 kernel, 8 NeuronCores.

Sharding (expert x F-half tensor parallel): core c handles expert e = c//2
and F-half = c%2 (columns [half*2048, (half+1)*2048) of w1, matching rows of
w2).  Each core computes, for every token routed to its expert,
    y_half = gelu(x @ w1[e, :, half]) @ w2[e, half, :]
with tokens living in the matmul FREE dimension, so the token count needs no
128-padding -- the compiled program uses C = max_e n_e token columns.  The
host gathers the two halves, scales by the routing probs, scatters back to
token order and adds the residual.

Precision: matmuls run in fp8e4m3 with MatmulPerfMode.DoubleRow (two
contraction k-tiles per instruction at 0.5 cycles/row).  Weights use
error-compensated quantization: w*32 ~= q8(w*32) + q8(w*32 - q8(w*32)), the
residual folded in as extra DoubleRow accumulation terms, so only the
activation quantization (x and gelu output) contributes first-order error.
Optionally (T1=3) the x residual dx = q8(x - q8(x)) is compensated too
(host-computed, zero extra device passes).  The *32 scaling keeps weight
values out of the fp8 subnormal range; it is undone for free via the gelu
activation scale (mm1) and the PSUM->SBUF copy scale (mm2).

This covers ANY routing map: every (token, expert) pair with routing_map
True lands in expert e's token list; tokens with 0 experts just pass the
residual through.  Capacity per expert is the full token count, so no
fallback path is needed.
"""
import math
import sys

import numpy as np

try:
    import concourse.bass as bass  # noqa: F401
except Exception:
    sys.path.insert(0, "/opt/trn_rl_repo")

import ml_dtypes

import concourse.bacc as bacc
import concourse.bass as bass
import concourse.mybir as mybir
import concourse.tile as tile
from concourse.bass_utils import run_bass_kernel_spmd

S, B, H, F, E = 1024, 2, 1024, 4096, 4
T = S * B
N_CORES = 8
FH = F // 2     # 2048, per-core F slice
NH = H // 128   # 8 h-chunks
NFH = FH // 128  # 16 f-chunks per core
F8_DT = mybir.dt.float8e4
F8_NP = ml_dtypes.float8_e4m3
WSCALE = 32.0   # weight pre-scale, undone on device

# mm1 terms: (w1s8, x8), (dw1s8, x8) [, (w1s8, dx8) if T1 == 3]
# mm2 terms: (w2s8, a8), (dw2s8, a8)
T1 = 2
T2 = 2

_NC_CACHE = {}


def _pieces(C):
    """Split C token columns into pieces of <= 512 (PSUM bank limit).

    First piece exactly 512 when possible (fp8 DMA runs >= 512B avoid the
    2x small-transfer latency penalty); the LAST piece is small (but >= 160
    so matmul exec stays above the 25ns sequencer dispatch) to shorten the
    post-matmul tail.
    """
    if C <= 512:
        widths = [C]
    else:
        last = max(160, min(512, round(C * 0.16)))
        mid = C - 512 - last
        if mid <= 0:
            widths = [512, C - 512] if C - 512 >= 160 else [C - 160, 160]
        else:
            P = math.ceil(mid / 512)
            base, rem = divmod(mid, P)
            widths = [512] + [base + 1] * rem + [base] * (P - rem) + [last]
    offs, o = [], 0
    for w in widths:
        offs.append(o)
        o += w
    return list(zip(offs, widths))


def _build_nc(C):
    key = (C,)
    if key in _NC_CACHE:
        return _NC_CACHE[key]
    pieces = _pieces(C)
    P = len(pieces)
    f32 = mybir.dt.float32
    gelu = mybir.ActivationFunctionType.Gelu
    dr = mybir.MatmulPerfMode.DoubleRow
    nc = bacc.Bacc("TRN2", target_bir_lowering=False, debug=False,
                   num_devices=N_CORES)
    xt_d = nc.declare_dram_parameter("xt8", [H, C], F8_DT, isOutput=False)
    if T1 == 3:
        dxt_d = nc.declare_dram_parameter("dxt8", [H, C], F8_DT,
                                          isOutput=False)
    # weights host-blocked, fq-major with the two quantization slots
    # interleaved, so one DMA covers a contiguous fq range for both slots
    # with >=1KB contiguous runs (runs < 512B pay 2x DMA latency).
    w1_d = nc.declare_dram_parameter("w1b", [NFH, 2, 128, NH, 128], F8_DT,
                                     isOutput=False)
    w2_d = nc.declare_dram_parameter("w2b", [NFH, 2, 128, H], F8_DT,
                                     isOutput=False)
    y_d = nc.declare_dram_parameter("y", [H, C], f32, isOutput=True)

    with tile.TileContext(nc) as tc:
        with (
            tc.tile_pool(name="resident", bufs=1) as rpool,
            tc.tile_pool(name="a", bufs=3) as apool,
            tc.tile_pool(name="y", bufs=2) as ypool,
            tc.tile_pool(name="pa", bufs=4, space="PSUM") as papool,
            tc.tile_pool(name="py", bufs=4, space="PSUM") as pypool,
        ):
            xt_sb = rpool.tile([128, NH, C], F8_DT, tag="xt")
            w1_sb = rpool.tile([128, NFH, 2, NH, 128], F8_DT, tag="w1")
            w2_sb = rpool.tile([128, NFH, 2, H], F8_DT, tag="w2")
            xt_ap = xt_d.ap().rearrange("(hc h) c -> h hc c", h=128)
            w1_ap = w1_d.ap().rearrange("q s h hc f -> h q s hc f")
            w2_ap = w2_d.ap().rearrange("q s f h -> f q s h")
            y_ap = y_d.ap().rearrange("(hc h) c -> h hc c", h=128)

            # Each dma_start holds the (serialized) HWDGE descriptor stage
            # ~625ns and the DMA engines for its transfer, so DMAs are few,
            # big, and issued in need order with geometric w1 head chunks so
            # the PE unblocks early.
            o0, w0 = pieces[0]
            if T1 == 3:
                dxt_sb = rpool.tile([128, NH, C], F8_DT, tag="dxt")
                dxt_ap = dxt_d.ap().rearrange("(hc h) c -> h hc c", h=128)

            def xt_piece(sb, ap, o, w, hcs):
                for (a, b) in hcs:
                    nc.sync.dma_start(sb[:, a:b, o:o + w],
                                      ap[:, a:b, o:o + w])

            def w_chunk(sb, ap, a, b):
                nc.sync.dma_start(sb[:, a:b], ap[:, a:b])

            xt_piece(xt_sb, xt_ap, o0, w0, [(0, 2)])
            nc.sync.dma_start(w1_sb[:, 0:1, 0], w1_ap[:, 0:1, 0])
            xt_piece(xt_sb, xt_ap, o0, w0, [(2, 4), (4, 8)])
            nc.sync.dma_start(w1_sb[:, 0:1, 1], w1_ap[:, 0:1, 1])
            if T1 == 3:
                xt_piece(dxt_sb, dxt_ap, o0, w0, [(0, 8)])
            for fq in range(1, NFH):
                w_chunk(w1_sb, w1_ap, fq, fq + 1)
            if len(pieces) > 1:
                o, w = pieces[1]
                xt_piece(xt_sb, xt_ap, o, w, [(0, 8)])
                if T1 == 3:
                    xt_piece(dxt_sb, dxt_ap, o, w, [(0, 8)])
            w_chunk(w2_sb, w2_ap, 0, 4)
            for (o, w) in pieces[2:]:
                xt_piece(xt_sb, xt_ap, o, w, [(0, 8)])
                if T1 == 3:
                    xt_piece(dxt_sb, dxt_ap, o, w, [(0, 8)])
            w_chunk(w2_sb, w2_ap, 4, 8)
            w_chunk(w2_sb, w2_ap, 8, 12)
            w_chunk(w2_sb, w2_ap, 12, NFH)

            # Warm the PE p-state during the DMA head: the cost model runs
            # the PE ~2x slower until it has been continuously busy for 3us,
            # so burn the input-DMA latency on self-contained dummy matmuls
            # over a memset tile instead of idling.
            warm_sb = rpool.tile([128, 2, 256], F8_DT, tag="warm")
            nc.vector.memset(warm_sb[:], 0)
            wpa = papool.tile([128, 256], f32, tag="pa", name="warm_pa")
            for _ in range(64):
                nc.tensor.matmul(wpa[:, :], warm_sb[:, :, 0:128],
                                 warm_sb[:, :, :], start=True, stop=True,
                                 perf_mode=dr)

            a_tiles = {}

            def mm1(p):
                off, W = pieces[p]
                a_sb = apool.tile([128, NFH, W], F8_DT, tag="a",
                                  name=f"a_{p}")
                a_tiles[p] = a_sb
                terms = [(0, xt_sb), (1, xt_sb)]
                if T1 == 3:
                    terms.append((0, dxt_sb))
                for fq in range(NFH):
                    pa = papool.tile([128, W], f32, tag="pa")
                    n = len(terms) * (NH // 2)
                    i = 0
                    for s, rhs in terms:
                        for j in range(NH // 2):
                            nc.tensor.matmul(
                                pa[:, :],
                                w1_sb[:, fq, s, 2 * j:2 * j + 2, :],
                                rhs[:, 2 * j:2 * j + 2, off:off + W],
                                start=(i == 0), stop=(i == n - 1),
                                perf_mode=dr)
                            i += 1
                    # psum holds 32*(x @ w1half); gelu(psum/32) -> fp8 a
                    nc.scalar.activation(a_sb[:, fq, :], pa[:, :], gelu,
                                         scale=1.0 / WSCALE)

            def mm2(p):
                off, W = pieces[p]
                last = (p == P - 1)
                a_sb = a_tiles.pop(p)
                y_sb = ypool.tile([128, NH, W], f32, tag="y", name=f"y_{p}")
                for hc in range(NH):
                    py = pypool.tile([128, W], f32, tag="py")
                    n = T2 * (NFH // 2)
                    i = 0
                    for s in range(T2):
                        for j in range(NFH // 2):
                            nc.tensor.matmul(
                                py[:, :],
                                w2_sb[:, 2 * j:2 * j + 2, s,
                                      hc * 128:(hc + 1) * 128],
                                a_sb[:, 2 * j:2 * j + 2, :],
                                start=(i == 0), stop=(i == n - 1),
                                perf_mode=dr)
                            i += 1
                    # psum holds 32*(a @ w2half); copy-with-scale undoes it
                    nc.vector.tensor_scalar_mul(y_sb[:, hc, :], py[:, :],
                                                1.0 / WSCALE)
                    # two y DMAs per piece (HWDGE is serialized, keep DMA
                    # count low, but don't clog the pipe right before the
                    # final small post-matmul DMA either).
                    if hc == 3:
                        nc.sync.dma_start(y_ap[:, :4, off:off + W],
                                          y_sb[:, :4, :])
                    elif hc == NH - 1:
                        nc.sync.dma_start(y_ap[:, 4:, off:off + W],
                                          y_sb[:, 4:, :])

            # Interleave so piece p's gelus fully overlap PE work, and the
            # PE never waits on the ACT engine at a piece boundary.
            mm1(0)
            for p in range(1, P):
                mm1(p)
                mm2(p - 1)
            mm2(P - 1)
    nc.compile()
    _NC_CACHE[key] = nc
    return nc


def _q8(v):
    return np.asarray(v, F8_NP)


def _block_w1(w):
    """[H, FH] -> [NFH, 128, NH, 128] (fq, h, hc, f)."""
    return w.reshape(NH, 128, NFH, 128).transpose(2, 1, 0, 3)


def kernel(hidden_states, mlp_residual, probs, routing_map, w1, w2,
           _trace=False):
    hidden_states = np.asarray(hidden_states, np.float32)
    mlp_residual = np.asarray(mlp_residual, np.float32)
    probs = np.asarray(probs, np.float32)
    routing_map = np.asarray(routing_map, bool)
    w1 = np.asarray(w1, np.float32)
    w2 = np.asarray(w2, np.float32)

    x = hidden_states.reshape(T, H)
    idx = [np.nonzero(routing_map[:, e])[0] for e in range(E)]
    C = max(1, max(len(i) for i in idx))

    nc = _build_nc(C)

    in_maps = []
    for c in range(N_CORES):
        e, half = divmod(c, 2)
        tok = idx[e]
        xtf = np.zeros((C, H), np.float32)
        if len(tok):
            xtf[:len(tok)] = x[tok]
        x8 = _q8(xtf)
        m = {"xt8": np.ascontiguousarray(x8.astype(np.float32).T).astype(
            F8_NP)}
        if T1 == 3:
            dx8 = _q8(xtf - x8.astype(np.float32))
            m["dxt8"] = np.ascontiguousarray(
                dx8.astype(np.float32).T).astype(F8_NP)
        w1s = w1[e, :, half * FH:(half + 1) * FH] * WSCALE
        w1s8 = _q8(w1s)
        dw1s8 = _q8(w1s - w1s8.astype(np.float32))
        # [NFH, 2, 128, NH, 128]: fq-major, quantization slots interleaved
        m["w1b"] = np.ascontiguousarray(np.stack(
            [_block_w1(w1s8.astype(np.float32)),
             _block_w1(dw1s8.astype(np.float32))], axis=1)).astype(F8_NP)
        w2s = w2[e, half * FH:(half + 1) * FH, :] * WSCALE
        w2s8 = _q8(w2s)
        dw2s8 = _q8(w2s - w2s8.astype(np.float32))
        # [NFH, 2, 128, H]
        m["w2b"] = np.ascontiguousarray(np.stack(
            [w2s8.astype(np.float32).reshape(NFH, 128, H),
             dw2s8.astype(np.float32).reshape(NFH, 128, H)],
            axis=1)).astype(F8_NP)
        in_maps.append(m)

    r = run_bass_kernel_spmd(nc, in_maps, list(range(N_CORES)), trace=_trace)

    out = mlp_residual.reshape(T, H).astype(np.float32).copy()
    for e in range(E):
        tok = idx[e]
        if len(tok) == 0:
            continue
        y = (np.asarray(r.results[2 * e]["y"][:, :len(tok)], np.float32)
             + np.asarray(r.results[2 * e + 1]["y"][:, :len(tok)],
                          np.float32))
        psel = probs[tok, e].astype(np.float32)
        out[tok] += (y * psel[None, :]).T
    result = out.reshape(S, B, H)
    if _trace:
        return result, r
    return result
